# revision 11
# baseline (speedup 1.0000x reference)
"""Trainium2 Bass kernel for nn_CustomLoss_69999376990919.

Math: the reference's A-inner-product modified Gram-Schmidt + projection
collapses to per-sample 4x4 Gram matrices
    G[s] = P_s diag(a_s) P_s^T,   R[s] = P_s diag(a_s) T_s
after which   loss = mean_s (4 - tr(R^T G^{-1} R)) / 4
(Cholesky of G == Gram-Schmidt in exact arithmetic; <v,Av> > 0 always holds
since coefficients > 0).  The device streams all inputs (memory-bound) and
produces G/R; the tiny 4x4 solves run on the host in float64.

Sharding: pure data parallelism, batch axis 0 split across 8 cores
(64 samples each).  Per core, samples run in 2 groups of 32 (bigger groups
amortize the ~60ns fixed cost of the per-matmul weight load best).
Layout: n = p*128 + f (p = SBUF partition, f = free chunk).  Per f-chunk,
a bf16 matmul pair accumulates G and R for all 32 samples jointly:
  lhsT = W(f) = (a*P)(f) as [128, (i,s)] stationary,
  rhs  = P(f) / T(f) as [128, (s,j)] moving,
  PSUM[(i,s), (s',j)] accumulated over the 128 f-chunks; the s==s' block
diagonals are the per-sample G/R entries (extracted on host).
Per group the DMAs are ordered p, a, t (SWDGE is FIFO) and the matmuls run
as a G-phase then an R-phase in t-half chunks, so compute starts as soon as
the group's predictions have landed and only the final R half-phase is
exposed after the last DMA.  bf16 is safe: the loss is 1 - O(1e-4);
bf16-quantized inputs move the final scalar by ~1e-9 relative.
"""

import os
from contextlib import ExitStack

import numpy as np

import concourse.bacc as bacc
import concourse.bass as bass
import concourse.tile as tile
from concourse import mybir
from concourse.bass_utils import run_bass_kernel_spmd

B, C, N = 512, 4, 16384
H = 0.0078125  # grid spacing; A = diag(h^2 * coefficients)
NCORES = 8
SPC = B // NCORES  # 64 samples per core
GS = 32            # samples per group
NG = SPC // GS     # 2 groups per core
P = 128            # SBUF partitions; n = p*128 + f
F = N // P         # 128 f-chunks
FH = F // 2        # f-half (t16/w16 tile granularity)
QP = C * GS        # psum partitions (i, s)

_CACHE = {}


def _build_bass():
    nc = bacc.Bacc(trn_type="TRN2")
    coeff = nc.dram_tensor("coeff", [SPC, N], mybir.dt.float32, kind="ExternalInput")
    preds = nc.dram_tensor("preds", [SPC, C, N], mybir.dt.float32, kind="ExternalInput")
    targs = nc.dram_tensor("targs", [SPC, N, C], mybir.dt.float32, kind="ExternalInput")
    out = nc.dram_tensor(
        "gr_out", [QP, NG * 2 * C * GS], mybir.dt.float32, kind="ExternalOutput"
    )

    coeff_v = coeff[:].rearrange("s (p f) -> p s f", p=P)
    preds_v = preds[:].rearrange("s j (p f) -> p s j f", p=P)
    targs_v = targs[:].rearrange("s (p f) m -> p s f m", p=P)

    with tile.TileContext(nc) as tc, ExitStack() as ctx:
        p16s = ctx.enter_context(tc.tile_pool(name="p16s", bufs=2))
        t16s = ctx.enter_context(tc.tile_pool(name="t16s", bufs=4))
        a16s = ctx.enter_context(tc.tile_pool(name="a16s", bufs=2))
        w16s = ctx.enter_context(tc.tile_pool(name="w16s", bufs=2))
        outs = ctx.enter_context(tc.tile_pool(name="outs", bufs=1))
        psums = ctx.enter_context(tc.tile_pool(name="psums", bufs=2, space="PSUM"))

        out_stage = outs.tile([QP, NG * 2 * C * GS], mybir.dt.float32)

        for g in range(NG):
            sl = slice(g * GS, (g + 1) * GS)

            # inputs cast fp32->bf16 during DMA (SWDGE, FIFO): predictions
            # first (they gate W and the G-phase), then coefficients, then
            # targets in f-halves so the R-phase can chase them
            p16 = p16s.tile([P, GS, C, F], mybir.dt.bfloat16, tag="p16")
            for x in range(2):  # split: bass caps DMA APs at 16384 descriptors
                sx = slice(g * GS + x * (GS // 2), g * GS + (x + 1) * (GS // 2))
                nc.gpsimd.dma_start(
                    out=p16[:, x * (GS // 2) : (x + 1) * (GS // 2), :, :],
                    in_=preds_v[:, sx, :, :],
                )

            a16 = a16s.tile([P, GS, F], mybir.dt.bfloat16, tag="a16")
            nc.gpsimd.dma_start(out=a16[:], in_=coeff_v[:, sl, :])

            t16 = []
            for h in range(2):
                th = t16s.tile(
                    [P, GS, FH, C], mybir.dt.bfloat16, tag="t16", name=f"t16_{g}_{h}"
                )
                nc.gpsimd.dma_start(
                    out=th[:], in_=targs_v[:, sl, h * FH : (h + 1) * FH, :]
                )
                t16.append(th)

            # W = a * p in bf16, layout [P, i, s, f]: f-contiguous DVE writes
            w16 = []
            for h in range(2):
                wh = w16s.tile(
                    [P, C, GS, FH], mybir.dt.bfloat16, tag="w16", name=f"w16_{g}_{h}"
                )
                for i in range(C):
                    nc.vector.tensor_mul(
                        wh[:, i, :, :],
                        a16[:, :, h * FH : (h + 1) * FH],
                        p16[:, :, i, h * FH : (h + 1) * FH],
                    )
                w16.append(wh)

            psum_g = psums.tile([QP, GS * C], mybir.dt.float32, tag="pg")
            psum_r = psums.tile([QP, GS * C], mybir.dt.float32, tag="pr")

            # G-phase: only needs p16 + W
            for f in range(F):
                h, fl = divmod(f, FH)
                nc.tensor.matmul(
                    psum_g[:],
                    w16[h][:, :, :, fl],   # [128, (i, s)] stationary
                    p16[:, :, :, f],       # [128, (s, j)] moving
                    start=(f == 0),
                    stop=(f == F - 1),
                )
            # R-phase: chases the two t16 half tiles
            for f in range(F):
                h, fl = divmod(f, FH)
                nc.tensor.matmul(
                    psum_r[:],
                    w16[h][:, :, :, fl],
                    t16[h][:, :, fl, :],   # [128, (s, m)] moving
                    start=(f == 0),
                    stop=(f == F - 1),
                )

            gw = 2 * C * GS  # out_stage columns per group
            nc.scalar.copy(
                out=out_stage[:, g * gw : g * gw + C * GS], in_=psum_g[:]
            )
            nc.scalar.copy(
                out=out_stage[:, g * gw + C * GS : (g + 1) * gw], in_=psum_r[:]
            )
            # drain this group's results while the next group computes
            nc.sync.dma_start(
                out=out[:, g * gw : (g + 1) * gw],
                in_=out_stage[:, g * gw : (g + 1) * gw],
            )

    if not nc.is_finalized():
        nc.finalize()
    return nc


def _get_nc():
    if "nc" not in _CACHE:
        _CACHE["nc"] = _build_bass()
    return _CACHE["nc"]


def kernel(coefficients, predictions, targets):
    co = np.ascontiguousarray(np.asarray(coefficients, dtype=np.float32))
    pr = np.ascontiguousarray(np.asarray(predictions, dtype=np.float32))
    tg = np.ascontiguousarray(np.asarray(targets, dtype=np.float32))
    assert co.shape == (B, N) and pr.shape == (B, C, N) and tg.shape == (B, N, C)

    nc = _get_nc()
    in_maps = []
    for c in range(NCORES):
        sl = slice(c * SPC, (c + 1) * SPC)
        in_maps.append({"coeff": co[sl], "preds": pr[sl], "targs": tg[sl]})

    res = run_bass_kernel_spmd(nc, in_maps, core_ids=list(range(NCORES)))
    _CACHE["last"] = res

    # host epilogue: extract per-sample 4x4 G/R block diagonals, fp64 solve
    G = np.empty((B, C, C), np.float64)
    R = np.empty((B, C, C), np.float64)
    gw = 2 * C * GS
    for c in range(NCORES):
        o = np.asarray(res.results[c]["gr_out"], dtype=np.float64)
        for g in range(NG):
            bg = o[:, g * gw : g * gw + C * GS].reshape(C, GS, GS, C)
            br = o[:, g * gw + C * GS : (g + 1) * gw].reshape(C, GS, GS, C)
            s0 = c * SPC + g * GS
            G[s0 : s0 + GS] = np.einsum("issj->sij", bg)
            R[s0 : s0 + GS] = np.einsum("issm->sim", br)

    G = 0.5 * (G + np.swapaxes(G, 1, 2))
    Xs = np.linalg.solve(G, R)
    val = (H * H) * np.einsum("bim,bim->b", R, Xs)
    loss = np.mean((4.0 - val) / 4.0)
    return np.float32(loss)
